# revision 10
# baseline (speedup 1.0000x reference)
"""ChebyKAN layer on 8 Trainium2 NeuronCores.

y = einsum('dbi,dio->bo', cheby_basis(tanh(x)), cheby_coeffs)

Strategy (per core, data-parallel over batch):
  - each core takes 1024 rows of x (8192/8) and the full coeffs
  - host prep: x pre-transposed ([i, b]); W[1:9] cast to bf16 (halves DMA
    and skips the on-device cast); the d=0 term is a constant in b, so
    bias[o] = sum_i W[0,i,o] is computed on host and added per-partition
    during PSUM evacuation — saves 1/9 of all matmul work
  - tanh on the scalar engine; Chebyshev recurrence in fp32 on the vector
    engine; bf16 copy of each T_d on the scalar engine
  - contraction as bf16 matmuls (full rate, fp32 PSUM accumulate):
    stationary = W[d, i-tile, o-tile], moving = T_d[i-tile, b-half],
    psum holds y.T chunks [o-tile 128, b-half 512] x 8 o-tiles = 8 banks
  - two b-halves of 512; W streamed from HBM once per half on the sync
    DMA queue; x + bias ride the vector queue so the tanh chain isn't
    stuck behind the W stream at startup
  - a few dummy matmuls on zeroed SBUF warm the PE clock (HAM) during
    the input ramp so real matmuls start at full rate
  - last degree runs o-tile-major so PSUM banks close progressively and
    evacuation + output DMA pipeline under the matmul stream
  - output is y.T per core; host transposes and concatenates
"""

import numpy as np
import ml_dtypes

import concourse.bass as bass
import concourse.tile as tile
from concourse import bacc, mybir
from concourse import bass_utils
from concourse.alu_op_type import AluOpType

N_CORES = 8
B = 8192
IC = 1024
OC = 1024
DEG = 8  # polynomial degree; degrees 1..8 matmul'd, degree 0 is the bias
BC = B // N_CORES  # 1024 batch rows per core
P = 128
NI = IC // P  # 8 i-tiles
NO = OC // P  # 8 o-tiles
BH = BC // 2  # 512, b-half
F32 = mybir.dt.float32
BF16 = mybir.dt.bfloat16

N_WARM_MM = 8  # dummy matmuls to lift the PE clock gate during the ramp

# W slab granularity (in i-tiles) per degree: the first degree of a half
# uses small slabs so the first matmul's W-DMA chain is short.
_D1_SLABS = [1, 1, 2, 2, 2]
_D_SLABS = [2, 2, 2, 2]
# x chunking (first_i_tile, n_i_tiles): small first chunks so the first
# tanh -> cast -> matmul chain starts ASAP
_X_CHUNKS_H0 = [(0, 1), (1, 1), (2, 2), (4, 2), (6, 2)]
_X_CHUNKS_H1 = [(0, 2), (2, 2), (4, 2), (6, 2)]


def _build(tanh_scale: float, tanh_bias: float):
    nc = bacc.Bacc("TRN2", target_bir_lowering=False, debug=False, num_devices=N_CORES)

    xT_d = nc.dram_tensor("xT", [IC, BC], F32, kind="ExternalInput").ap()
    w_d = nc.dram_tensor("w8", [DEG, IC, OC], BF16, kind="ExternalInput").ap()
    bias_d = nc.dram_tensor("biasT", [P, NO], F32, kind="ExternalInput").ap()
    yt_d = nc.dram_tensor("yt", [OC, BC], F32, kind="ExternalOutput").ap()

    with tile.TileContext(nc) as tc:
        with (
            tc.tile_pool(name="const", bufs=1) as constp,
            tc.tile_pool(name="xin", bufs=3) as xinp,
            tc.tile_pool(name="xt", bufs=2) as xtp,
            tc.tile_pool(name="state", bufs=3) as statep,
            tc.tile_pool(name="prod", bufs=2) as prodp,
            tc.tile_pool(name="tr", bufs=2) as trp,
            tc.tile_pool(name="wstage", bufs=9) as wstagep,
            tc.tile_pool(name="evac", bufs=4) as evacp,
            tc.tile_pool(name="ps", bufs=8, space=bass.MemorySpace.PSUM) as psp,
        ):
            # PE warm-up: zeroed operands, own psum tile (aliases a real
            # bank later; every real accumulation opens with start=True so
            # the garbage never leaks)
            warm = constp.tile([P, BH], BF16)
            nc.vector.memset(warm[:], 0.0)
            warm_ps = psp.tile([P, BH], F32, tag="ps", name="warm_ps")
            for k in range(N_WARM_MM):
                nc.tensor.matmul(
                    warm_ps[:], warm[:, :P], warm[:], start=True, stop=True
                )

            bias_sb = constp.tile([P, NO], F32)
            nc.gpsimd.dma_start(bias_sb[:], bias_d[:, :])

            def emit_w_slabs(h, d, slab_sizes, it0=0, tag=None):
                """DMA W[d] (d in 1..8) i-tile slabs as bf16; returns
                [(first_it, ntiles, wst_tile), ...]."""
                out = []
                for ws, nt in enumerate(slab_sizes):
                    wst = wstagep.tile(
                        [P, nt * OC], BF16, tag="wstage", name=f"wst_{h}_{tag or d}_{ws}_{it0}"
                    )
                    nc.sync.dma_start(
                        wst[:].rearrange("p (il o) -> p il o", il=nt),
                        w_d[d - 1, it0 * P : (it0 + nt) * P, :].rearrange(
                            "(il p) o -> p il o", p=P
                        ),
                    )
                    out.append((it0, nt, wst))
                    it0 += nt
                return out

            def emit_matmuls(accs, wr_slabs, d, tr_d):
                if d == DEG:
                    # whole-degree ot-major: each acc's accumulation closes
                    # early so psum banks free progressively — evac + output
                    # DMA pipeline under the matmul stream
                    for ot in range(NO):
                        for it0, nt, wr in wr_slabs:
                            for il in range(nt):
                                it = it0 + il
                                rhs = tr_d[:, it * BH : (it + 1) * BH]
                                nc.tensor.matmul(
                                    accs[ot][:],
                                    wr[:, il * OC + ot * P : il * OC + (ot + 1) * P],
                                    rhs,
                                    start=False,
                                    stop=(it == NI - 1),
                                )
                    return
                for it0, nt, wr in wr_slabs:
                    for il in range(nt):
                        it = it0 + il
                        rhs = tr_d[:, it * BH : (it + 1) * BH]
                        for ot in range(NO):
                            nc.tensor.matmul(
                                accs[ot][:],
                                wr[:, il * OC + ot * P : il * OC + (ot + 1) * P],
                                rhs,
                                start=(d == 1 and it == 0),
                                stop=False,
                            )

            # ---- x.T load + tanh -> fp32 xt, emitted per half ----
            # xt free layout: i_tile-major, 512 b-local each; x rides the
            # vector DMA queue (separate from the W stream). If cast_to is
            # given, each tanh chunk is immediately followed by its bf16
            # cast on the scalar engine (T_1 = xt).
            def emit_xt(h, chunks, cast_to=None):
                xt = xtp.tile([P, NI * BH], F32, tag="xt", name=f"xt_{h}")
                for it0, ntile in chunks:
                    xst = xinp.tile(
                        [P, ntile * BH], F32, tag="xin", name=f"xs_{h}_{it0}"
                    )
                    nc.gpsimd.dma_start(
                        xst[:].rearrange("p (il b) -> p il b", il=ntile),
                        xT_d[it0 * P : (it0 + ntile) * P, h * BH : (h + 1) * BH].rearrange(
                            "(il p) b -> p il b", p=P
                        ),
                    )
                    sl = slice(it0 * BH, (it0 + ntile) * BH)
                    nc.scalar.activation(
                        xt[:, sl],
                        xst[:],
                        mybir.ActivationFunctionType.Tanh,
                        bias=tanh_bias,
                        scale=tanh_scale,
                    )
                    if cast_to is not None:
                        nc.scalar.activation(
                            cast_to[:, sl], xt[:, sl], mybir.ActivationFunctionType.Copy
                        )
                return xt

            xts = [None, None]
            tr1s = [trp.tile([P, NI * BH], BF16, tag="tr", name="tr_0_1"), None]

            for h in range(2):
                if h == 0:
                    # first small W slab ahead of the x stream so the PE can
                    # start as soon as tr_1 chunk0 is ready; x next; remaining
                    # d1 W after
                    head = emit_w_slabs(0, 1, _D1_SLABS[:2])
                    xts[0] = emit_xt(0, _X_CHUNKS_H0, cast_to=tr1s[0])
                    d1_slabs_h0 = head + emit_w_slabs(
                        0, 1, _D1_SLABS[2:], it0=2, tag="1b"
                    )
                xt = xts[h]
                # ---- accumulation psum tiles: y.T chunk per o-tile ----
                accs = [
                    psp.tile([P, BH], F32, tag="ps", name=f"acc_h{h}_o{ot}")
                    for ot in range(NO)
                ]

                # ---- degree loop ----
                t_m1 = xt  # T_{d-1} (fp32 slab)
                t_m2 = None  # T_{d-2}
                for d in range(1, DEG + 1):
                    # bf16 moving operand for this degree
                    if d == 1:
                        # h0: casts interleaved with tanh; h1: casts emitted
                        # during h0's last degree (ahead of h0's evacuation
                        # in the scalar FIFO)
                        tr_d = tr1s[h]
                    else:
                        t_new = statep.tile(
                            [P, NI * BH], F32, tag="state", name=f"st_{h}_{d}"
                        )
                        tr_d = trp.tile([P, NI * BH], BF16, tag="tr", name=f"tr_{h}_{d}")
                        QS = NI * BH // 4
                        for q in range(4):
                            sl = slice(q * QS, (q + 1) * QS)
                            prod = prodp.tile(
                                [P, QS], F32, tag="prod", name=f"prod_{h}_{d}_{q}"
                            )
                            nc.vector.scalar_tensor_tensor(
                                prod[:],
                                t_m1[:, sl],
                                2.0,
                                xt[:, sl],
                                AluOpType.mult,
                                AluOpType.mult,
                            )
                            if d == 2:
                                # T2 = 2*xt^2 - 1
                                nc.vector.tensor_scalar_sub(t_new[:, sl], prod[:], 1.0)
                            else:
                                nc.vector.tensor_sub(t_new[:, sl], prod[:], t_m2[:, sl])
                            nc.scalar.activation(
                                tr_d[:, sl],
                                t_new[:, sl],
                                mybir.ActivationFunctionType.Copy,
                            )
                        t_m2, t_m1 = t_m1, t_new
                    if d == 1:
                        t_m2, t_m1 = xt, xt  # T1 = xt; T0 handled via scalar sub at d=2

                    # ---- W stream + matmuls for this degree ----
                    if h == 0 and d == 1:
                        wr_slabs = d1_slabs_h0
                    else:
                        wr_slabs = emit_w_slabs(h, d, _D1_SLABS if d == 1 else _D_SLABS)
                    emit_matmuls(accs, wr_slabs, d, tr_d)
                    if h == 0 and d == 2:
                        xts[1] = emit_xt(1, _X_CHUNKS_H1)
                    if h == 0 and d == DEG:
                        # h1's T_1 cast, emitted now so it lands ahead of
                        # h0's evacuation in the scalar FIFO and runs during
                        # the d8 matmul stream
                        tr1s[1] = trp.tile(
                            [P, NI * BH], BF16, tag="tr", name="tr_1_1"
                        )
                        QS = NI * BH // 4
                        for q in range(4):
                            sl = slice(q * QS, (q + 1) * QS)
                            nc.scalar.activation(
                                tr1s[1][:, sl],
                                xts[1][:, sl],
                                mybir.ActivationFunctionType.Copy,
                            )

                # ---- evacuate psum (+ degree-0 bias) -> SBUF -> y.T ----
                for ot in range(NO):
                    ev = evacp.tile([P, BH], F32, tag="evac", name=f"ev_h{h}_o{ot}")
                    if ot % 2 == 0:
                        nc.vector.tensor_scalar_add(
                            ev[:], accs[ot][:], bias_sb[:, ot : ot + 1]
                        )
                    else:
                        nc.scalar.activation(
                            ev[:],
                            accs[ot][:],
                            mybir.ActivationFunctionType.Identity,
                            bias=bias_sb[:, ot : ot + 1],
                        )
                    nc.scalar.dma_start(
                        yt_d[ot * P : (ot + 1) * P, h * BH : (h + 1) * BH],
                        ev[:],
                    )

    nc.compile()
    return nc


_CACHE: dict = {}


def make_in_maps(x, w):
    w8 = np.ascontiguousarray(w[1 : DEG + 1]).astype(ml_dtypes.bfloat16)
    # bias[o] = sum_i W[0,i,o], laid out [P, NO] so column ot is the
    # per-partition bias for o-tile ot
    bias = w[0].astype(np.float64).sum(axis=0).astype(np.float32)
    biasT = np.ascontiguousarray(bias.reshape(NO, P).T)
    return [
        {
            "xT": np.ascontiguousarray(x[c * BC : (c + 1) * BC].T),
            "w8": w8,
            "biasT": biasT,
        }
        for c in range(N_CORES)
    ]


def kernel(x, cheby_coeffs, tanh_scale, tanh_bias):
    x = np.ascontiguousarray(np.asarray(x, dtype=np.float32))
    w = np.ascontiguousarray(np.asarray(cheby_coeffs, dtype=np.float32))
    ts = float(np.asarray(tanh_scale))
    tb = float(np.asarray(tanh_bias))

    key = (ts, tb)
    if key not in _CACHE:
        _CACHE[key] = _build(ts, tb)
    nc = _CACHE[key]

    in_maps = make_in_maps(x, w)
    res = bass_utils.run_bass_kernel_spmd(
        nc, in_maps, core_ids=list(range(N_CORES)), trace=False
    )

    y = np.empty((B, OC), dtype=np.float32)
    for c in range(N_CORES):
        y[c * BC : (c + 1) * BC, :] = res.results[c]["yt"].T
    return y


# revision 12
# speedup vs baseline: 1.1949x; 1.1949x over previous
"""ChebyKAN layer on 8 Trainium2 NeuronCores.

y = einsum('dbi,dio->bo', cheby_basis(tanh(x)), cheby_coeffs)

Strategy (per core, data-parallel over batch):
  - each core takes 1024 rows of x (8192/8) and the full coeffs
  - host prep: x pre-transposed ([i, b]); W[1:9] cast to bf16 (halves DMA
    and skips the on-device cast); the d=0 term is a constant in b, so
    bias[o] = sum_i W[0,i,o] is computed on host and added per-partition
    during PSUM evacuation — saves 1/9 of all matmul work
  - tanh on the scalar engine; Chebyshev recurrence in fp32 on the vector
    engine; bf16 copy of each T_d on the scalar engine
  - contraction as bf16 matmuls (full rate, fp32 PSUM accumulate):
    stationary = W[d, i-tile, o-tile], moving = T_d[i-tile, b-half],
    psum holds y.T chunks [o-tile 128, b-half 512] x 8 o-tiles = 8 banks
  - two b-halves of 512; W streamed from HBM once per half on the sync
    DMA queue; x + bias ride the vector queue so the tanh chain isn't
    stuck behind the W stream at startup
  - a few dummy matmuls on zeroed SBUF warm the PE clock (HAM) during
    the input ramp so real matmuls start at full rate
  - last degree runs o-tile-major so PSUM banks close progressively and
    evacuation + output DMA pipeline under the matmul stream
  - output is y.T per core; host transposes and concatenates
"""

import numpy as np
import ml_dtypes

import concourse.bass as bass
import concourse.tile as tile
from concourse import bacc, mybir
from concourse import bass_utils
from concourse.alu_op_type import AluOpType

N_CORES = 8
B = 8192
IC = 1024
OC = 1024
DEG = 8  # polynomial degree; degrees 1..8 matmul'd, degree 0 is the bias
BC = B // N_CORES  # 1024 batch rows per core
P = 128
NI = IC // P  # 8 i-tiles
NO = OC // P  # 8 o-tiles
BH = BC // 2  # 512, b-half
F32 = mybir.dt.float32
BF16 = mybir.dt.bfloat16

N_WARM_MM = 8  # dummy matmuls to lift the PE clock gate during the ramp

# W slab granularity (in i-tiles) per degree: the first degree of a half
# uses small slabs so the first matmul's W-DMA chain is short.
_D1_SLABS = [1, 1, 2, 2, 2]
_D_SLABS = [2, 2, 2, 2]
# x chunking (first_i_tile, n_i_tiles): small first chunks so the first
# tanh -> cast -> matmul chain starts ASAP
_X_CHUNKS_H0 = [(0, 1), (1, 1), (2, 2), (4, 2), (6, 2)]
_X_CHUNKS_H1 = [(0, 2), (2, 2), (4, 2), (6, 2)]


def _build(tanh_scale: float, tanh_bias: float):
    nc = bacc.Bacc("TRN2", target_bir_lowering=False, debug=False, num_devices=N_CORES)

    xT_d = nc.dram_tensor("xT", [IC, BC], F32, kind="ExternalInput").ap()
    w_d = nc.dram_tensor("w8", [DEG, IC, OC], BF16, kind="ExternalInput").ap()
    bias_d = nc.dram_tensor("biasT", [P, NO], F32, kind="ExternalInput").ap()
    yt_d = nc.dram_tensor("yt", [OC, BC], F32, kind="ExternalOutput").ap()

    with tile.TileContext(nc) as tc:
        with (
            tc.tile_pool(name="const", bufs=1) as constp,
            tc.tile_pool(name="xin", bufs=3) as xinp,
            tc.tile_pool(name="xt", bufs=2) as xtp,
            tc.tile_pool(name="state", bufs=3) as statep,
            tc.tile_pool(name="prod", bufs=2) as prodp,
            tc.tile_pool(name="tr", bufs=2) as trp,
            tc.tile_pool(name="wstage", bufs=9) as wstagep,
            tc.tile_pool(name="evac", bufs=4) as evacp,
            tc.tile_pool(name="ps", bufs=8, space=bass.MemorySpace.PSUM) as psp,
        ):
            # PE warm-up: zeroed operands, own psum tile (aliases a real
            # bank later; every real accumulation opens with start=True so
            # the garbage never leaks)
            warm = constp.tile([P, BH], BF16)
            nc.vector.memset(warm[:], 0.0)
            warm_ps = psp.tile([P, BH], F32, tag="ps", name="warm_ps")
            for k in range(N_WARM_MM):
                nc.tensor.matmul(
                    warm_ps[:], warm[:, :P], warm[:], start=True, stop=True
                )

            bias_sb = constp.tile([P, NO], F32)
            nc.gpsimd.dma_start(bias_sb[:], bias_d[:, :])

            def emit_w_slabs(h, d, slab_sizes, it0=0, tag=None):
                """DMA W[d] (d in 1..8) i-tile slabs as bf16; returns
                [(first_it, ntiles, wst_tile), ...]."""
                out = []
                for ws, nt in enumerate(slab_sizes):
                    wst = wstagep.tile(
                        [P, nt * OC], BF16, tag="wstage", name=f"wst_{h}_{tag or d}_{ws}_{it0}"
                    )
                    nc.sync.dma_start(
                        wst[:].rearrange("p (il o) -> p il o", il=nt),
                        w_d[d - 1, it0 * P : (it0 + nt) * P, :].rearrange(
                            "(il p) o -> p il o", p=P
                        ),
                    )
                    out.append((it0, nt, wst))
                    it0 += nt
                return out

            def emit_matmuls(accs, wr_slabs, d, tr_d):
                if d == DEG:
                    # whole-degree ot-major: each acc's accumulation closes
                    # early so psum banks free progressively — evac + output
                    # DMA pipeline under the matmul stream
                    for ot in range(NO):
                        for it0, nt, wr in wr_slabs:
                            for il in range(nt):
                                it = it0 + il
                                rhs = tr_d[:, it * BH : (it + 1) * BH]
                                nc.tensor.matmul(
                                    accs[ot][:],
                                    wr[:, il * OC + ot * P : il * OC + (ot + 1) * P],
                                    rhs,
                                    start=False,
                                    stop=(it == NI - 1),
                                )
                    return
                for it0, nt, wr in wr_slabs:
                    for il in range(nt):
                        it = it0 + il
                        rhs = tr_d[:, it * BH : (it + 1) * BH]
                        for ot in range(NO):
                            nc.tensor.matmul(
                                accs[ot][:],
                                wr[:, il * OC + ot * P : il * OC + (ot + 1) * P],
                                rhs,
                                start=(d == 1 and it == 0),
                                stop=False,
                            )

            # ---- x.T load + tanh -> fp32 xt ----
            # xt free layout: i_tile-major, 512 b-local each. One chunk =
            # DMA + tanh (+ optional immediate bf16 cast, T_1 = xt).
            def emit_x_chunk(h, xt, it0, ntile, cast_to=None):
                xst = xinp.tile(
                    [P, ntile * BH], F32, tag="xin", name=f"xs_{h}_{it0}"
                )
                nc.sync.dma_start(
                    xst[:].rearrange("p (il b) -> p il b", il=ntile),
                    xT_d[it0 * P : (it0 + ntile) * P, h * BH : (h + 1) * BH].rearrange(
                        "(il p) b -> p il b", p=P
                    ),
                )
                sl = slice(it0 * BH, (it0 + ntile) * BH)
                nc.scalar.activation(
                    xt[:, sl],
                    xst[:],
                    mybir.ActivationFunctionType.Tanh,
                    bias=tanh_bias,
                    scale=tanh_scale,
                )
                if cast_to is not None:
                    nc.scalar.activation(
                        cast_to[:, sl], xt[:, sl], mybir.ActivationFunctionType.Copy
                    )

            def emit_xt(h, chunks, cast_to=None):
                xt = xtp.tile([P, NI * BH], F32, tag="xt", name=f"xt_{h}")
                for it0, ntile in chunks:
                    emit_x_chunk(h, xt, it0, ntile, cast_to)
                return xt

            xts = [None, None]
            tr1s = [trp.tile([P, NI * BH], BF16, tag="tr", name="tr_0_1"), None]

            for h in range(2):
                if h == 0:
                    # interleave x chunks and W d1 slabs on the sync DMA
                    # ring so the tanh -> cast -> first-matmul chain starts
                    # ASAP while the first W tiles stream alongside
                    xts[0] = xtp.tile([P, NI * BH], F32, tag="xt", name="xt_0")
                    emit_x_chunk(0, xts[0], 0, 1, cast_to=tr1s[0])
                    head = emit_w_slabs(0, 1, [1])
                    emit_x_chunk(0, xts[0], 1, 1, cast_to=tr1s[0])
                    head += emit_w_slabs(0, 1, [1], it0=1, tag="1a")
                    emit_x_chunk(0, xts[0], 2, 2, cast_to=tr1s[0])
                    emit_x_chunk(0, xts[0], 4, 2, cast_to=tr1s[0])
                    emit_x_chunk(0, xts[0], 6, 2, cast_to=tr1s[0])
                    d1_slabs_h0 = head + emit_w_slabs(
                        0, 1, _D1_SLABS[2:], it0=2, tag="1b"
                    )
                xt = xts[h]
                # ---- accumulation psum tiles: y.T chunk per o-tile ----
                accs = [
                    psp.tile([P, BH], F32, tag="ps", name=f"acc_h{h}_o{ot}")
                    for ot in range(NO)
                ]

                # ---- degree loop ----
                t_m1 = xt  # T_{d-1} (fp32 slab)
                t_m2 = None  # T_{d-2}
                for d in range(1, DEG + 1):
                    # bf16 moving operand for this degree
                    if d == 1:
                        # h0: casts interleaved with tanh; h1: casts emitted
                        # during h0's last degree (ahead of h0's evacuation
                        # in the scalar FIFO)
                        tr_d = tr1s[h]
                    else:
                        t_new = statep.tile(
                            [P, NI * BH], F32, tag="state", name=f"st_{h}_{d}"
                        )
                        tr_d = trp.tile([P, NI * BH], BF16, tag="tr", name=f"tr_{h}_{d}")
                        QS = NI * BH // 4
                        for q in range(4):
                            sl = slice(q * QS, (q + 1) * QS)
                            prod = prodp.tile(
                                [P, QS], F32, tag="prod", name=f"prod_{h}_{d}_{q}"
                            )
                            nc.vector.scalar_tensor_tensor(
                                prod[:],
                                t_m1[:, sl],
                                2.0,
                                xt[:, sl],
                                AluOpType.mult,
                                AluOpType.mult,
                            )
                            if d == 2:
                                # T2 = 2*xt^2 - 1
                                nc.vector.tensor_scalar_sub(t_new[:, sl], prod[:], 1.0)
                            else:
                                nc.vector.tensor_sub(t_new[:, sl], prod[:], t_m2[:, sl])
                            nc.scalar.activation(
                                tr_d[:, sl],
                                t_new[:, sl],
                                mybir.ActivationFunctionType.Copy,
                            )
                        t_m2, t_m1 = t_m1, t_new
                    if d == 1:
                        t_m2, t_m1 = xt, xt  # T1 = xt; T0 handled via scalar sub at d=2

                    # ---- W stream + matmuls for this degree ----
                    if h == 0 and d == 1:
                        wr_slabs = d1_slabs_h0
                    else:
                        wr_slabs = emit_w_slabs(h, d, _D1_SLABS if d == 1 else _D_SLABS)
                    emit_matmuls(accs, wr_slabs, d, tr_d)
                    if h == 0 and d == 2:
                        xts[1] = emit_xt(1, _X_CHUNKS_H1)
                    if h == 0 and d == DEG:
                        # h1's T_1 cast, emitted now so it lands ahead of
                        # h0's evacuation in the scalar FIFO and runs during
                        # the d8 matmul stream
                        tr1s[1] = trp.tile(
                            [P, NI * BH], BF16, tag="tr", name="tr_1_1"
                        )
                        QS = NI * BH // 4
                        for q in range(4):
                            sl = slice(q * QS, (q + 1) * QS)
                            nc.scalar.activation(
                                tr1s[1][:, sl],
                                xts[1][:, sl],
                                mybir.ActivationFunctionType.Copy,
                            )

                # ---- evacuate psum (+ degree-0 bias) -> SBUF -> y.T ----
                for ot in range(NO):
                    ev = evacp.tile([P, BH], F32, tag="evac", name=f"ev_h{h}_o{ot}")
                    if ot % 2 == 0:
                        nc.vector.tensor_scalar_add(
                            ev[:], accs[ot][:], bias_sb[:, ot : ot + 1]
                        )
                    else:
                        nc.scalar.activation(
                            ev[:],
                            accs[ot][:],
                            mybir.ActivationFunctionType.Identity,
                            bias=bias_sb[:, ot : ot + 1],
                        )
                    nc.scalar.dma_start(
                        yt_d[ot * P : (ot + 1) * P, h * BH : (h + 1) * BH],
                        ev[:],
                    )

    nc.compile()
    return nc


_CACHE: dict = {}


def make_in_maps(x, w):
    w8 = np.ascontiguousarray(w[1 : DEG + 1]).astype(ml_dtypes.bfloat16)
    # bias[o] = sum_i W[0,i,o], laid out [P, NO] so column ot is the
    # per-partition bias for o-tile ot
    bias = w[0].astype(np.float64).sum(axis=0).astype(np.float32)
    biasT = np.ascontiguousarray(bias.reshape(NO, P).T)
    return [
        {
            "xT": np.ascontiguousarray(x[c * BC : (c + 1) * BC].T),
            "w8": w8,
            "biasT": biasT,
        }
        for c in range(N_CORES)
    ]


def kernel(x, cheby_coeffs, tanh_scale, tanh_bias):
    x = np.ascontiguousarray(np.asarray(x, dtype=np.float32))
    w = np.ascontiguousarray(np.asarray(cheby_coeffs, dtype=np.float32))
    ts = float(np.asarray(tanh_scale))
    tb = float(np.asarray(tanh_bias))

    key = (ts, tb)
    if key not in _CACHE:
        _CACHE[key] = _build(ts, tb)
    nc = _CACHE[key]

    in_maps = make_in_maps(x, w)
    res = bass_utils.run_bass_kernel_spmd(
        nc, in_maps, core_ids=list(range(N_CORES)), trace=False
    )

    y = np.empty((B, OC), dtype=np.float32)
    for c in range(N_CORES):
        y[c * BC : (c + 1) * BC, :] = res.results[c]["yt"].T
    return y


# revision 15
# speedup vs baseline: 1.2173x; 1.0187x over previous
"""ChebyKAN layer on 8 Trainium2 NeuronCores.

y = einsum('dbi,dio->bo', cheby_basis(tanh(x)), cheby_coeffs)

Strategy (per core, data-parallel over batch):
  - each core takes 1024 rows of x (8192/8) and the full coeffs
  - host prep: x pre-transposed ([i, b]); W[1:9] cast to bf16 (halves DMA
    and skips the on-device cast); the d=0 term is a constant in b, so
    bias[o] = sum_i W[0,i,o] is computed on host and added per-partition
    during PSUM evacuation — saves 1/9 of all matmul work
  - tanh on the scalar engine; Chebyshev recurrence in fp32 on the vector
    engine; bf16 copy of each T_d on the scalar engine
  - contraction as bf16 matmuls (full rate, fp32 PSUM accumulate):
    stationary = W[d, i-tile, o-tile], moving = T_d[i-tile, b-half],
    psum holds y.T chunks [o-tile 128, b-half 512] x 8 o-tiles = 8 banks
  - two b-halves of 512; W streamed from HBM once per half on the sync
    DMA queue; x + bias ride the vector queue so the tanh chain isn't
    stuck behind the W stream at startup
  - a few dummy matmuls on zeroed SBUF warm the PE clock (HAM) during
    the input ramp so real matmuls start at full rate
  - last degree runs o-tile-major so PSUM banks close progressively and
    evacuation + output DMA pipeline under the matmul stream
  - output is y.T per core; host transposes and concatenates
"""

import numpy as np
import ml_dtypes

import concourse.bass as bass
import concourse.tile as tile
from concourse import bacc, mybir
from concourse import bass_utils
from concourse.alu_op_type import AluOpType

N_CORES = 8
B = 8192
IC = 1024
OC = 1024
DEG = 8  # polynomial degree; degrees 1..8 matmul'd, degree 0 is the bias
BC = B // N_CORES  # 1024 batch rows per core
P = 128
NI = IC // P  # 8 i-tiles
NO = OC // P  # 8 o-tiles
BH = BC // 2  # 512, b-half
F32 = mybir.dt.float32
BF16 = mybir.dt.bfloat16

N_WARM_MM = 8  # dummy matmuls to lift the PE clock gate during the ramp

# W slab granularity (in i-tiles) per degree: the first degree of a half
# uses small slabs so the first matmul's W-DMA chain is short.
_D1_SLABS = [1, 1, 2, 2, 2]
_D_SLABS = [2, 2, 2, 2]
# x chunking (first_i_tile, n_i_tiles): small first chunks so the first
# tanh -> cast -> matmul chain starts ASAP
_X_CHUNKS_H0 = [(0, 1), (1, 1), (2, 2), (4, 2), (6, 2)]
_X_CHUNKS_H1 = [(0, 2), (2, 2), (4, 2), (6, 2)]


def _build(tanh_scale: float, tanh_bias: float):
    nc = bacc.Bacc("TRN2", target_bir_lowering=False, debug=False, num_devices=N_CORES)

    xT_d = nc.dram_tensor("xT", [IC, BC], F32, kind="ExternalInput").ap()
    w_d = nc.dram_tensor("w8", [DEG, IC, OC], BF16, kind="ExternalInput").ap()
    bias_d = nc.dram_tensor("biasT", [P, NO], F32, kind="ExternalInput").ap()
    yt_d = nc.dram_tensor("yt", [OC, BC], F32, kind="ExternalOutput").ap()

    with tile.TileContext(nc) as tc:
        with (
            tc.tile_pool(name="const", bufs=1) as constp,
            tc.tile_pool(name="xin", bufs=3) as xinp,
            tc.tile_pool(name="xt", bufs=2) as xtp,
            tc.tile_pool(name="state", bufs=3) as statep,
            tc.tile_pool(name="prod", bufs=2) as prodp,
            tc.tile_pool(name="tr", bufs=2) as trp,
            tc.tile_pool(name="tr1b", bufs=1) as tr1bp,
            tc.tile_pool(name="wstage", bufs=9) as wstagep,
            tc.tile_pool(name="evac", bufs=4) as evacp,
            tc.tile_pool(name="ps", bufs=8, space=bass.MemorySpace.PSUM) as psp,
        ):
            # PE warm-up: zeroed operands, own psum tile (aliases a real
            # bank later; every real accumulation opens with start=True so
            # the garbage never leaks)
            warm = constp.tile([P, BH], BF16)
            nc.vector.memset(warm[:], 0.0)
            warm_ps = psp.tile([P, BH], F32, tag="ps", name="warm_ps")
            for k in range(N_WARM_MM):
                nc.tensor.matmul(
                    warm_ps[:], warm[:, :P], warm[:], start=True, stop=True
                )

            bias_sb = constp.tile([P, NO], F32)
            nc.gpsimd.dma_start(bias_sb[:], bias_d[:, :])

            def emit_w_slabs(h, d, slab_sizes, it0=0, tag=None):
                """DMA W[d] (d in 1..8) i-tile slabs as bf16; returns
                [(first_it, ntiles, wst_tile), ...]."""
                out = []
                for ws, nt in enumerate(slab_sizes):
                    wst = wstagep.tile(
                        [P, nt * OC], BF16, tag="wstage", name=f"wst_{h}_{tag or d}_{ws}_{it0}"
                    )
                    nc.sync.dma_start(
                        wst[:].rearrange("p (il o) -> p il o", il=nt),
                        w_d[d - 1, it0 * P : (it0 + nt) * P, :].rearrange(
                            "(il p) o -> p il o", p=P
                        ),
                    )
                    out.append((it0, nt, wst))
                    it0 += nt
                return out

            def emit_matmuls(accs, wr_slabs, d, tr_d):
                if d == DEG:
                    # whole-degree ot-major: each acc's accumulation closes
                    # early so psum banks free progressively — evac + output
                    # DMA pipeline under the matmul stream
                    for ot in range(NO):
                        for it0, nt, wr in wr_slabs:
                            for il in range(nt):
                                it = it0 + il
                                rhs = tr_d[:, it * BH : (it + 1) * BH]
                                nc.tensor.matmul(
                                    accs[ot][:],
                                    wr[:, il * OC + ot * P : il * OC + (ot + 1) * P],
                                    rhs,
                                    start=False,
                                    stop=(it == NI - 1),
                                )
                    return
                for it0, nt, wr in wr_slabs:
                    for il in range(nt):
                        it = it0 + il
                        rhs = tr_d[:, it * BH : (it + 1) * BH]
                        for ot in range(NO):
                            nc.tensor.matmul(
                                accs[ot][:],
                                wr[:, il * OC + ot * P : il * OC + (ot + 1) * P],
                                rhs,
                                start=(d == 1 and it == 0),
                                stop=False,
                            )

            # ---- x.T load + tanh ----
            # xt free layout: i_tile-major, 512 b-local each. One chunk =
            # DMA + bf16 tanh straight into T_1 (feeds the matmuls, no cast
            # step on the critical path) + f32 tanh into xt (feeds the
            # recurrence).
            def emit_x_chunk(h, xt, tr1, it0, ntile, ring=None):
                xst = xinp.tile(
                    [P, ntile * BH], F32, tag="xin", name=f"xs_{h}_{it0}"
                )
                (ring or nc.sync).dma_start(
                    xst[:].rearrange("p (il b) -> p il b", il=ntile),
                    xT_d[it0 * P : (it0 + ntile) * P, h * BH : (h + 1) * BH].rearrange(
                        "(il p) b -> p il b", p=P
                    ),
                )
                sl = slice(it0 * BH, (it0 + ntile) * BH)
                nc.scalar.activation(
                    tr1[:, sl],
                    xst[:],
                    mybir.ActivationFunctionType.Tanh,
                    bias=tanh_bias,
                    scale=tanh_scale,
                )
                nc.scalar.activation(
                    xt[:, sl],
                    xst[:],
                    mybir.ActivationFunctionType.Tanh,
                    bias=tanh_bias,
                    scale=tanh_scale,
                )

            xts = [None, None]
            tr1s = [
                trp.tile([P, NI * BH], BF16, tag="tr", name="tr_0_1"),
                tr1bp.tile([P, NI * BH], BF16, tag="tr1b", name="tr_1_1"),
            ]

            for h in range(2):
                if h == 0:
                    # x chunk 0 rides the scalar ring (activates earliest);
                    # the rest interleave with W d1 slabs on the sync ring so
                    # the tanh -> first-matmul chain starts ASAP while the
                    # first W tiles stream alongside
                    xts[0] = xtp.tile([P, NI * BH], F32, tag="xt", name="xt_0")
                    emit_x_chunk(0, xts[0], tr1s[0], 0, 1, ring=nc.scalar)
                    head = emit_w_slabs(0, 1, [1])
                    emit_x_chunk(0, xts[0], tr1s[0], 1, 1)
                    head += emit_w_slabs(0, 1, [1], it0=1, tag="1a")
                    emit_x_chunk(0, xts[0], tr1s[0], 2, 2)
                    emit_x_chunk(0, xts[0], tr1s[0], 4, 2)
                    emit_x_chunk(0, xts[0], tr1s[0], 6, 2)
                    d1_slabs_h0 = head + emit_w_slabs(
                        0, 1, _D1_SLABS[2:], it0=2, tag="1b"
                    )
                xt = xts[h]
                # ---- accumulation psum tiles: y.T chunk per o-tile ----
                accs = [
                    psp.tile([P, BH], F32, tag="ps", name=f"acc_h{h}_o{ot}")
                    for ot in range(NO)
                ]

                # ---- degree loop ----
                t_m1 = xt  # T_{d-1} (fp32 slab)
                t_m2 = None  # T_{d-2}
                for d in range(1, DEG + 1):
                    # bf16 moving operand for this degree
                    if d == 1:
                        # h0: casts interleaved with tanh; h1: casts emitted
                        # during h0's last degree (ahead of h0's evacuation
                        # in the scalar FIFO)
                        tr_d = tr1s[h]
                    else:
                        t_new = statep.tile(
                            [P, NI * BH], F32, tag="state", name=f"st_{h}_{d}"
                        )
                        tr_d = trp.tile([P, NI * BH], BF16, tag="tr", name=f"tr_{h}_{d}")
                        QS = NI * BH // 4
                        for q in range(4):
                            sl = slice(q * QS, (q + 1) * QS)
                            prod = prodp.tile(
                                [P, QS], F32, tag="prod", name=f"prod_{h}_{d}_{q}"
                            )
                            nc.vector.scalar_tensor_tensor(
                                prod[:],
                                t_m1[:, sl],
                                2.0,
                                xt[:, sl],
                                AluOpType.mult,
                                AluOpType.mult,
                            )
                            if d == 2:
                                # T2 = 2*xt^2 - 1
                                nc.vector.tensor_scalar_sub(t_new[:, sl], prod[:], 1.0)
                            else:
                                nc.vector.tensor_sub(t_new[:, sl], prod[:], t_m2[:, sl])
                            nc.scalar.activation(
                                tr_d[:, sl],
                                t_new[:, sl],
                                mybir.ActivationFunctionType.Copy,
                            )
                        t_m2, t_m1 = t_m1, t_new
                    if d == 1:
                        t_m2, t_m1 = xt, xt  # T1 = xt; T0 handled via scalar sub at d=2

                    # ---- W stream + matmuls for this degree ----
                    if h == 0 and d == 1:
                        wr_slabs = d1_slabs_h0
                    else:
                        wr_slabs = emit_w_slabs(h, d, _D1_SLABS if d == 1 else _D_SLABS)
                    emit_matmuls(accs, wr_slabs, d, tr_d)
                    if h == 0 and d == 2:
                        xts[1] = xtp.tile([P, NI * BH], F32, tag="xt", name="xt_1")
                        for it0, ntile in _X_CHUNKS_H1:
                            emit_x_chunk(1, xts[1], tr1s[1], it0, ntile)

                # ---- evacuate psum (+ degree-0 bias) -> SBUF -> y.T ----
                for ot in range(NO):
                    ev = evacp.tile([P, BH], F32, tag="evac", name=f"ev_h{h}_o{ot}")
                    if ot % 2 == 0:
                        nc.vector.tensor_scalar_add(
                            ev[:], accs[ot][:], bias_sb[:, ot : ot + 1]
                        )
                    else:
                        nc.scalar.activation(
                            ev[:],
                            accs[ot][:],
                            mybir.ActivationFunctionType.Identity,
                            bias=bias_sb[:, ot : ot + 1],
                        )
                    nc.scalar.dma_start(
                        yt_d[ot * P : (ot + 1) * P, h * BH : (h + 1) * BH],
                        ev[:],
                    )

    nc.compile()
    return nc


_CACHE: dict = {}


def make_in_maps(x, w):
    w8 = np.ascontiguousarray(w[1 : DEG + 1]).astype(ml_dtypes.bfloat16)
    # bias[o] = sum_i W[0,i,o], laid out [P, NO] so column ot is the
    # per-partition bias for o-tile ot
    bias = w[0].astype(np.float64).sum(axis=0).astype(np.float32)
    biasT = np.ascontiguousarray(bias.reshape(NO, P).T)
    return [
        {
            "xT": np.ascontiguousarray(x[c * BC : (c + 1) * BC].T),
            "w8": w8,
            "biasT": biasT,
        }
        for c in range(N_CORES)
    ]


def kernel(x, cheby_coeffs, tanh_scale, tanh_bias):
    x = np.ascontiguousarray(np.asarray(x, dtype=np.float32))
    w = np.ascontiguousarray(np.asarray(cheby_coeffs, dtype=np.float32))
    ts = float(np.asarray(tanh_scale))
    tb = float(np.asarray(tanh_bias))

    key = (ts, tb)
    if key not in _CACHE:
        _CACHE[key] = _build(ts, tb)
    nc = _CACHE[key]

    in_maps = make_in_maps(x, w)
    res = bass_utils.run_bass_kernel_spmd(
        nc, in_maps, core_ids=list(range(N_CORES)), trace=False
    )

    y = np.empty((B, OC), dtype=np.float32)
    for c in range(N_CORES):
        y[c * BC : (c + 1) * BC, :] = res.results[c]["yt"].T
    return y


# revision 18
# speedup vs baseline: 1.2210x; 1.0030x over previous
"""ChebyKAN layer on 8 Trainium2 NeuronCores.

y = einsum('dbi,dio->bo', cheby_basis(tanh(x)), cheby_coeffs)

Strategy (per core, data-parallel over batch):
  - each core takes 1024 rows of x (8192/8) and the full coeffs
  - host prep: x pre-transposed ([i, b]); W[1:9] cast to bf16 (halves DMA
    and skips the on-device cast); the d=0 term is a constant in b, so
    bias[o] = sum_i W[0,i,o] is computed on host and added per-partition
    during PSUM evacuation — saves 1/9 of all matmul work
  - tanh on the scalar engine; Chebyshev recurrence in fp32 on the vector
    engine; bf16 copy of each T_d on the scalar engine
  - contraction as bf16 matmuls (full rate, fp32 PSUM accumulate):
    stationary = W[d, i-tile, o-tile], moving = T_d[i-tile, b-half],
    psum holds y.T chunks [o-tile 128, b-half 512] x 8 o-tiles = 8 banks
  - two b-halves of 512; W streamed from HBM once per half on the sync
    DMA queue; x + bias ride the vector queue so the tanh chain isn't
    stuck behind the W stream at startup
  - a few dummy matmuls on zeroed SBUF warm the PE clock (HAM) during
    the input ramp so real matmuls start at full rate
  - last degree runs o-tile-major so PSUM banks close progressively and
    evacuation + output DMA pipeline under the matmul stream
  - output is y.T per core; host transposes and concatenates
"""

import numpy as np
import ml_dtypes

import concourse.bass as bass
import concourse.tile as tile
from concourse import bacc, mybir
from concourse import bass_utils
from concourse.alu_op_type import AluOpType

N_CORES = 8
B = 8192
IC = 1024
OC = 1024
DEG = 8  # polynomial degree; degrees 1..8 matmul'd, degree 0 is the bias
BC = B // N_CORES  # 1024 batch rows per core
P = 128
NI = IC // P  # 8 i-tiles
NO = OC // P  # 8 o-tiles
BH = BC // 2  # 512, b-half
F32 = mybir.dt.float32
BF16 = mybir.dt.bfloat16

N_WARM_MM = 11  # dummy matmuls to lift the PE clock gate during the ramp

# W slab granularity (in i-tiles) per degree: the first degree of a half
# uses small slabs so the first matmul's W-DMA chain is short.
_D1_SLABS = [1, 1, 2, 2, 2]
_D_SLABS = [2, 2, 2, 2]
# x chunking for half 1 (first_i_tile, n_i_tiles); half 0's chunks are
# inlined with the W-slab interleave at startup
_X_CHUNKS_H1 = [(0, 2), (2, 2), (4, 2), (6, 2)]


def _build(tanh_scale: float, tanh_bias: float):
    nc = bacc.Bacc("TRN2", target_bir_lowering=False, debug=False, num_devices=N_CORES)

    xT_d = nc.dram_tensor("xT", [IC, BC], F32, kind="ExternalInput").ap()
    w_d = nc.dram_tensor("w8", [DEG, IC, OC], BF16, kind="ExternalInput").ap()
    bias_d = nc.dram_tensor("biasT", [P, NO], F32, kind="ExternalInput").ap()
    yt_d = nc.dram_tensor("yt", [OC, BC], F32, kind="ExternalOutput").ap()

    with tile.TileContext(nc) as tc:
        with (
            tc.tile_pool(name="const", bufs=1) as constp,
            tc.tile_pool(name="xin", bufs=3) as xinp,
            tc.tile_pool(name="xt", bufs=2) as xtp,
            tc.tile_pool(name="state", bufs=3) as statep,
            tc.tile_pool(name="prod", bufs=2) as prodp,
            tc.tile_pool(name="tr", bufs=2) as trp,
            tc.tile_pool(name="tr1b", bufs=1) as tr1bp,
            tc.tile_pool(name="wstage", bufs=9) as wstagep,
            tc.tile_pool(name="evac", bufs=4) as evacp,
            tc.tile_pool(name="ps", bufs=8, space=bass.MemorySpace.PSUM) as psp,
        ):
            # PE warm-up: zeroed operands, own psum tile (aliases a real
            # bank later; every real accumulation opens with start=True so
            # the garbage never leaks)
            warm = constp.tile([P, BH], BF16)
            nc.gpsimd.memset(warm[:], 0.0)
            warm_ps = psp.tile([P, BH], F32, tag="ps", name="warm_ps")
            for k in range(N_WARM_MM):
                nc.tensor.matmul(
                    warm_ps[:], warm[:, :P], warm[:], start=True, stop=True
                )

            bias_sb = constp.tile([P, NO], F32)
            nc.gpsimd.dma_start(bias_sb[:], bias_d[:, :])

            def emit_w_slabs(h, d, slab_sizes, it0=0, tag=None):
                """DMA W[d] (d in 1..8) i-tile slabs as bf16; returns
                [(first_it, ntiles, wst_tile), ...]."""
                out = []
                for ws, nt in enumerate(slab_sizes):
                    wst = wstagep.tile(
                        [P, nt * OC], BF16, tag="wstage", name=f"wst_{h}_{tag or d}_{ws}_{it0}"
                    )
                    nc.sync.dma_start(
                        wst[:].rearrange("p (il o) -> p il o", il=nt),
                        w_d[d - 1, it0 * P : (it0 + nt) * P, :].rearrange(
                            "(il p) o -> p il o", p=P
                        ),
                    )
                    out.append((it0, nt, wst))
                    it0 += nt
                return out

            def emit_matmuls(accs, wr_slabs, d, tr_d):
                if d == DEG:
                    # whole-degree ot-major: each acc's accumulation closes
                    # early so psum banks free progressively — evac + output
                    # DMA pipeline under the matmul stream
                    for ot in range(NO):
                        for it0, nt, wr in wr_slabs:
                            for il in range(nt):
                                it = it0 + il
                                rhs = tr_d[:, it * BH : (it + 1) * BH]
                                nc.tensor.matmul(
                                    accs[ot][:],
                                    wr[:, il * OC + ot * P : il * OC + (ot + 1) * P],
                                    rhs,
                                    start=False,
                                    stop=(it == NI - 1),
                                )
                    return
                for it0, nt, wr in wr_slabs:
                    for il in range(nt):
                        it = it0 + il
                        rhs = tr_d[:, it * BH : (it + 1) * BH]
                        for ot in range(NO):
                            nc.tensor.matmul(
                                accs[ot][:],
                                wr[:, il * OC + ot * P : il * OC + (ot + 1) * P],
                                rhs,
                                start=(d == 1 and it == 0),
                                stop=False,
                            )

            # ---- x.T load + tanh ----
            # xt free layout: i_tile-major, 512 b-local each. One chunk =
            # DMA + bf16 tanh straight into T_1 (feeds the matmuls, no cast
            # step on the critical path) + f32 tanh into xt (feeds the
            # recurrence).
            def emit_x_chunk(h, xt, tr1, it0, ntile, ring=None):
                xst = xinp.tile(
                    [P, ntile * BH], F32, tag="xin", name=f"xs_{h}_{it0}"
                )
                (ring or nc.sync).dma_start(
                    xst[:].rearrange("p (il b) -> p il b", il=ntile),
                    xT_d[it0 * P : (it0 + ntile) * P, h * BH : (h + 1) * BH].rearrange(
                        "(il p) b -> p il b", p=P
                    ),
                )
                sl = slice(it0 * BH, (it0 + ntile) * BH)
                nc.scalar.activation(
                    tr1[:, sl],
                    xst[:],
                    mybir.ActivationFunctionType.Tanh,
                    bias=tanh_bias,
                    scale=tanh_scale,
                )
                nc.scalar.activation(
                    xt[:, sl],
                    xst[:],
                    mybir.ActivationFunctionType.Tanh,
                    bias=tanh_bias,
                    scale=tanh_scale,
                )

            xts = [None, None]
            tr1s = [
                trp.tile([P, NI * BH], BF16, tag="tr", name="tr_0_1"),
                tr1bp.tile([P, NI * BH], BF16, tag="tr1b", name="tr_1_1"),
            ]

            for h in range(2):
                if h == 0:
                    # x chunk 0 rides the scalar ring (activates earliest);
                    # the rest interleave with W d1 slabs on the sync ring so
                    # the tanh -> first-matmul chain starts ASAP while the
                    # first W tiles stream alongside
                    xts[0] = xtp.tile([P, NI * BH], F32, tag="xt", name="xt_0")
                    emit_x_chunk(0, xts[0], tr1s[0], 0, 1, ring=nc.scalar)
                    head = emit_w_slabs(0, 1, [1])
                    emit_x_chunk(0, xts[0], tr1s[0], 1, 1)
                    head += emit_w_slabs(0, 1, [1], it0=1, tag="1a")
                    emit_x_chunk(0, xts[0], tr1s[0], 2, 2)
                    emit_x_chunk(0, xts[0], tr1s[0], 4, 2)
                    emit_x_chunk(0, xts[0], tr1s[0], 6, 2)
                    d1_slabs_h0 = head + emit_w_slabs(
                        0, 1, _D1_SLABS[2:], it0=2, tag="1b"
                    )
                xt = xts[h]
                # ---- accumulation psum tiles: y.T chunk per o-tile ----
                accs = [
                    psp.tile([P, BH], F32, tag="ps", name=f"acc_h{h}_o{ot}")
                    for ot in range(NO)
                ]

                # ---- degree loop ----
                t_m1 = xt  # T_{d-1} (fp32 slab)
                t_m2 = None  # T_{d-2}
                for d in range(1, DEG + 1):
                    # bf16 moving operand for this degree
                    if d == 1:
                        # h0: casts interleaved with tanh; h1: casts emitted
                        # during h0's last degree (ahead of h0's evacuation
                        # in the scalar FIFO)
                        tr_d = tr1s[h]
                    else:
                        t_new = statep.tile(
                            [P, NI * BH], F32, tag="state", name=f"st_{h}_{d}"
                        )
                        tr_d = trp.tile([P, NI * BH], BF16, tag="tr", name=f"tr_{h}_{d}")
                        QS = NI * BH // 4
                        for q in range(4):
                            sl = slice(q * QS, (q + 1) * QS)
                            prod = prodp.tile(
                                [P, QS], F32, tag="prod", name=f"prod_{h}_{d}_{q}"
                            )
                            nc.vector.scalar_tensor_tensor(
                                prod[:],
                                t_m1[:, sl],
                                2.0,
                                xt[:, sl],
                                AluOpType.mult,
                                AluOpType.mult,
                            )
                            if d == 2:
                                # T2 = 2*xt^2 - 1
                                nc.vector.tensor_scalar_sub(t_new[:, sl], prod[:], 1.0)
                            else:
                                nc.vector.tensor_sub(t_new[:, sl], prod[:], t_m2[:, sl])
                            nc.scalar.activation(
                                tr_d[:, sl],
                                t_new[:, sl],
                                mybir.ActivationFunctionType.Copy,
                            )
                        t_m2, t_m1 = t_m1, t_new
                    if d == 1:
                        t_m2, t_m1 = xt, xt  # T1 = xt; T0 handled via scalar sub at d=2

                    # ---- W stream + matmuls for this degree ----
                    if h == 0 and d == 1:
                        wr_slabs = d1_slabs_h0
                    else:
                        wr_slabs = emit_w_slabs(h, d, _D1_SLABS if d == 1 else _D_SLABS)
                    emit_matmuls(accs, wr_slabs, d, tr_d)
                    if h == 0 and d == 2:
                        xts[1] = xtp.tile([P, NI * BH], F32, tag="xt", name="xt_1")
                        for it0, ntile in _X_CHUNKS_H1:
                            emit_x_chunk(1, xts[1], tr1s[1], it0, ntile)

                # ---- evacuate psum (+ degree-0 bias) -> SBUF -> y.T ----
                for ot in range(NO):
                    ev = evacp.tile([P, BH], F32, tag="evac", name=f"ev_h{h}_o{ot}")
                    if ot % 2 == 0:
                        nc.vector.tensor_scalar_add(
                            ev[:], accs[ot][:], bias_sb[:, ot : ot + 1]
                        )
                    else:
                        nc.scalar.activation(
                            ev[:],
                            accs[ot][:],
                            mybir.ActivationFunctionType.Identity,
                            bias=bias_sb[:, ot : ot + 1],
                        )
                    nc.scalar.dma_start(
                        yt_d[ot * P : (ot + 1) * P, h * BH : (h + 1) * BH],
                        ev[:],
                    )

    nc.compile()
    return nc


_CACHE: dict = {}


def make_in_maps(x, w):
    w8 = np.ascontiguousarray(w[1 : DEG + 1]).astype(ml_dtypes.bfloat16)
    # bias[o] = sum_i W[0,i,o], laid out [P, NO] so column ot is the
    # per-partition bias for o-tile ot
    bias = w[0].astype(np.float64).sum(axis=0).astype(np.float32)
    biasT = np.ascontiguousarray(bias.reshape(NO, P).T)
    return [
        {
            "xT": np.ascontiguousarray(x[c * BC : (c + 1) * BC].T),
            "w8": w8,
            "biasT": biasT,
        }
        for c in range(N_CORES)
    ]


def kernel(x, cheby_coeffs, tanh_scale, tanh_bias):
    x = np.ascontiguousarray(np.asarray(x, dtype=np.float32))
    w = np.ascontiguousarray(np.asarray(cheby_coeffs, dtype=np.float32))
    ts = float(np.asarray(tanh_scale))
    tb = float(np.asarray(tanh_bias))

    key = (ts, tb)
    if key not in _CACHE:
        _CACHE[key] = _build(ts, tb)
    nc = _CACHE[key]

    in_maps = make_in_maps(x, w)
    res = bass_utils.run_bass_kernel_spmd(
        nc, in_maps, core_ids=list(range(N_CORES)), trace=False
    )

    y = np.empty((B, OC), dtype=np.float32)
    for c in range(N_CORES):
        y[c * BC : (c + 1) * BC, :] = res.results[c]["yt"].T
    return y
